# revision 2
# baseline (speedup 1.0000x reference)
"""Trainium2 Bass kernel for nn_Conv2d: x[32,128,56,56] * W[256,128,3,3] + b -> [32,256,56,56].

Stride 1, padding 1, dilation 1. Data-parallel over batch across 8 NeuronCores
(4 images per core, no collectives). Per core the conv is one accumulation
group of 9 matmuls per output tile (one per kernel tap):
PSUM[cout_chunk=128, R*56] += matmul(lhsT=Wt[tap][cin, cout_chunk],
rhs=shifted window of the zero-padded input row-block [cin=128, R+2, 58]).
Bias is fused into the PSUM->SBUF drain on the scalar engine.

Self-contained: hardcodes shapes; host-side pre-pads/retiles x and
pre-transposes W so every device DMA is contiguous.
"""

import numpy as np

B, CIN, H, W_ = 32, 128, 56, 56
COUT, KH, KW = 256, 3, 3
NCORES = 8
BPC = B // NCORES          # images per core
R = 8                      # output rows per tile -> matmul free dim R*56 = 448
NT = H // R                # row tiles per image
NPIX = R * W_              # 448
HP, WP = H + 2, W_ + 2     # padded 58x58

# "float32" = exact fp32 (4 cycles/row on PE). "float32r" = TF32-like
# single-pass mode (1 cycle/row at N>=256, ~1e-4 absmax relative error).
MM_DTYPE = "float32r"

_cache = {}


def _build(mm_dtype_name):
    import concourse.mybir as mybir
    import concourse.tile as tile
    from concourse import bacc

    dt = mybir.dt
    mmdt = getattr(dt, mm_dtype_name)

    nc = bacc.Bacc("TRN2", target_bir_lowering=False, debug=False)

    # x arrives host-pre-padded per row-tile: [image, row_tile, cin, R+2, 58]
    # (zero border baked in, halo rows duplicated) so every x DMA is one
    # fully contiguous 290KB copy and the kernel needs no memsets.
    x_d = nc.dram_tensor(
        "x", [BPC, NT, CIN, R + 2, WP], mmdt, kind="ExternalInput"
    )
    # [chunk, cin, tap, cout_slice]: one contiguous 0.59MB DMA per cout chunk
    wt_d = nc.dram_tensor(
        "wt", [COUT // 128, CIN, KH * KW, 128], mmdt, kind="ExternalInput"
    )
    b_d = nc.dram_tensor("bias", [128, COUT // 128], dt.float32, kind="ExternalInput")
    o_d = nc.dram_tensor("out", [BPC, COUT, H, W_], dt.float32, kind="ExternalOutput")

    with tile.TileContext(nc) as tc:
        with (
            tc.tile_pool(name="const", bufs=1) as const_pool,
            tc.tile_pool(name="xin", bufs=1) as xin_pool,
            tc.tile_pool(name="outp", bufs=4) as out_pool,
            tc.tile_pool(name="psum", bufs=4, space="PSUM") as psum_pool,
        ):
            # One input tile per (image, row-tile): rows h0-1..h0+R of the
            # padded image (R+2 rows x 58 cols). Separate logical tiles keep
            # Tile's dependency tracking fine-grained: the first matmul group
            # only waits on its own ~290KB DMA, not all of x. Halo rows are
            # duplicated host-side (25% extra x traffic; DMA is far from the
            # bottleneck). All BPC*NT tiles stay resident (~65KB/partition).
            xt = {}

            def load_x_tile(n, ht):
                t = xin_pool.tile([CIN, R + 2, WP], mmdt, tag=f"x{n}_{ht}")
                xt[(n, ht)] = t
                nc.sync.dma_start(t[:], x_d[n, ht])

            # DMA issue order tracks the first matmul group's critical path:
            # first x tile, then chunk-0 weights, then everything else.
            load_x_tile(0, 0)
            w_t = const_pool.tile([CIN, COUT // 128, KH * KW, 128], mmdt)
            nc.sync.dma_start(w_t[:, 0], wt_d[0])
            load_x_tile(0, 1)
            nc.sync.dma_start(w_t[:, 1], wt_d[1])
            b_t = const_pool.tile([128, COUT // 128], dt.float32)
            nc.sync.dma_start(b_t[:], b_d[:])
            for n in range(BPC):
                for ht in range(NT):
                    if (n, ht) not in xt:
                        load_x_tile(n, ht)

            for n in range(BPC):
                for ht in range(NT):
                    t = xt[(n, ht)]
                    for c in range(COUT // 128):
                        p = psum_pool.tile([128, R, W_], dt.float32, tag="ps")
                        for kh in range(KH):
                            for kw in range(KW):
                                pos = kh * KW + kw
                                nc.tensor.matmul(
                                    p[:],
                                    w_t[:, c, pos],
                                    t[:, kh : kh + R, kw : kw + W_],
                                    start=(pos == 0),
                                    stop=(pos == KH * KW - 1),
                                )
                        ot = out_pool.tile([128, R, W_], dt.float32, tag="ot")
                        nc.scalar.activation(
                            ot[:],
                            p[:],
                            mybir.ActivationFunctionType.Identity,
                            bias=b_t[:, c : c + 1],
                        )
                        nc.sync.dma_start(
                            o_d[n, c * 128 : (c + 1) * 128, ht * R : ht * R + R, :],
                            ot[:],
                        )

    nc.compile()
    return nc


def _make_in_maps(x, W, b):
    x = np.asarray(x, dtype=np.float32)
    W = np.asarray(W, dtype=np.float32)
    b = np.asarray(b, dtype=np.float32)

    # Pre-pad and re-tile x: [B, CIN, 56, 56] -> [B, NT, CIN, R+2, 58] where
    # row-tile ht holds padded rows h0..h0+R+1 (zero border baked in).
    xpad = np.zeros((B, CIN, HP, WP), dtype=np.float32)
    xpad[:, :, 1 : H + 1, 1 : W_ + 1] = x
    xt = np.empty((B, NT, CIN, R + 2, WP), dtype=np.float32)
    for ht in range(NT):
        xt[:, ht] = xpad[:, :, ht * R : ht * R + R + 2, :]

    # [cout, cin, kh, kw] -> [cout_chunk, cin, kh*kw, cout_slice], contiguous
    wt = np.ascontiguousarray(
        W.reshape(COUT // 128, 128, CIN, KH * KW).transpose(0, 2, 3, 1)
    )
    bh = np.ascontiguousarray(b.reshape(COUT // 128, 128).T)

    return [
        {
            "x": xt[core * BPC : (core + 1) * BPC],
            "wt": wt,
            "bias": bh,
        }
        for core in range(NCORES)
    ]


def kernel(x, W, b):
    from concourse.bass_utils import run_bass_kernel_spmd

    if MM_DTYPE not in _cache:
        _cache[MM_DTYPE] = _build(MM_DTYPE)
    nc = _cache[MM_DTYPE]

    in_maps = _make_in_maps(x, W, b)
    try:
        res = run_bass_kernel_spmd(nc, in_maps, list(range(NCORES))).results
    except Exception:
        # A prior session can leave the accelerator in a transient
        # unrecoverable state; one retry after re-init clears it.
        import time

        time.sleep(15)
        res = run_bass_kernel_spmd(nc, in_maps, list(range(NCORES))).results
    return np.concatenate([res[i]["out"] for i in range(NCORES)], axis=0)



# revision 4
# speedup vs baseline: 1.1489x; 1.1489x over previous
"""Trainium2 Bass kernel for nn_Conv2d: x[32,128,56,56] * W[256,128,3,3] + b -> [32,256,56,56].

Stride 1, padding 1, dilation 1. Data-parallel over batch across 8 NeuronCores
(4 images per core, no collectives). Per core the conv is matmul accumulation
over the 9 kernel taps: PSUM[cout_chunk=128, R*56] += W_tap[cin, cout].T @
(shifted window of the zero-padded input row-block [cin=128, R+2, 58]).

Operands are bf16 (PSUM accumulation stays fp32): same 1 PE-cycle/row as
fp32r but halves input DMA and, crucially, allows a standalone LDWEIGHTS.
The loop is weight-stationary: at each (row_tile, cout_chunk) one LDWEIGHTS
per tap feeds matmuls for all 4 images (4 live PSUM banks), cutting the
per-matmul 128-row weight-load overhead 4x. Bias is fused into the
PSUM->SBUF drain on the scalar engine.

Self-contained: hardcodes shapes; host-side pre-pads/retiles x and
pre-transposes W so every device DMA is contiguous.
"""

import numpy as np

B, CIN, H, W_ = 32, 128, 56, 56
COUT, KH, KW = 256, 3, 3
NCORES = 8
BPC = B // NCORES          # images per core (= weight-stationary group size)
R = 8                      # output rows per tile -> matmul free dim R*56 = 448
NT = H // R                # row tiles per image
HP, WP = H + 2, W_ + 2     # padded 58x58
NCH = COUT // 128          # cout chunks

# True: standalone LDWEIGHTS per (tap, chunk) + non-self-loading matmuls
# (ldweights=False). False: every matmul reloads the PE array.
LDW_SKIP = True

_cache = {}


def _build(ldw_skip):
    import concourse.mybir as mybir
    import concourse.tile as tile
    from concourse import bacc

    dt = mybir.dt

    nc = bacc.Bacc("TRN2", target_bir_lowering=False, debug=False)

    # x arrives host-pre-padded per row-tile, all 4 images interleaved:
    # [row_tile, cin, image, R+2, 58] (zero border baked in, halo rows
    # duplicated) so each row-tile is ONE contiguous 594KB DMA with 4.6KB
    # per-partition descriptors.
    x_d = nc.dram_tensor(
        "x", [NT, CIN, BPC, R + 2, WP], dt.bfloat16, kind="ExternalInput"
    )
    # [chunk, cin, tap, cout_slice]: one contiguous DMA per cout chunk
    wt_d = nc.dram_tensor(
        "wt", [NCH, CIN, KH * KW, 128], dt.bfloat16, kind="ExternalInput"
    )
    b_d = nc.dram_tensor("bias", [128, NCH], dt.float32, kind="ExternalInput")
    o_d = nc.dram_tensor("out", [BPC, COUT, H, W_], dt.float32, kind="ExternalOutput")

    with tile.TileContext(nc) as tc:
        with (
            tc.tile_pool(name="const", bufs=1) as const_pool,
            tc.tile_pool(name="xin", bufs=1) as xin_pool,
            tc.tile_pool(name="outp", bufs=4) as out_pool,
            tc.tile_pool(name="psum", bufs=8, space="PSUM") as psum_pool,
        ):
            # All NT row-tiles stay resident (~33KB/partition at bf16).
            xt = {}

            def load_x_tile(ht):
                t = xin_pool.tile([CIN, BPC, R + 2, WP], dt.bfloat16, tag=f"x{ht}")
                xt[ht] = t
                nc.sync.dma_start(t[:], x_d[ht])

            # DMA issue order tracks the first group's critical path.
            load_x_tile(0)
            w_t = const_pool.tile([CIN, NCH, KH * KW, 128], dt.bfloat16)
            nc.sync.dma_start(w_t[:, 0], wt_d[0])
            nc.sync.dma_start(w_t[:, 1], wt_d[1])
            b_t = const_pool.tile([128, NCH], dt.float32)
            nc.sync.dma_start(b_t[:], b_d[:])
            for ht in range(1, NT):
                load_x_tile(ht)

            for ht in range(NT):
                t = xt[ht]
                for c in range(NCH):
                    ps = [
                        psum_pool.tile([128, R, W_], dt.float32, tag="ps", name="ps")
                        for _ in range(BPC)
                    ]
                    for kh in range(KH):
                        for kw in range(KW):
                            pos = kh * KW + kw
                            if ldw_skip:
                                nc.tensor.ldweights(w_t[:, c, pos])
                            for n in range(BPC):
                                mm = nc.tensor.matmul(
                                    ps[n][:],
                                    w_t[:, c, pos],
                                    t[:, n, kh : kh + R, kw : kw + W_],
                                    start=(pos == 0),
                                    stop=(pos == KH * KW - 1),
                                )
                                if ldw_skip:
                                    mm.ins.ldweights = False
                    for n in range(BPC):
                        ot = out_pool.tile([128, R, W_], dt.float32, tag="ot")
                        nc.scalar.activation(
                            ot[:],
                            ps[n][:],
                            mybir.ActivationFunctionType.Identity,
                            bias=b_t[:, c : c + 1],
                        )
                        nc.sync.dma_start(
                            o_d[n, c * 128 : (c + 1) * 128, ht * R : ht * R + R, :],
                            ot[:],
                        )

    nc.compile()
    return nc


def _make_in_maps(x, W, b):
    import ml_dtypes

    bf16 = ml_dtypes.bfloat16
    x = np.asarray(x, dtype=np.float32)
    W = np.asarray(W, dtype=np.float32)
    b = np.asarray(b, dtype=np.float32)

    # Pre-pad and re-tile x: [B, CIN, 56, 56] -> per-core [NT, CIN, BPC, R+2, 58]
    # where row-tile ht holds padded rows ht*R..ht*R+R+1 (zero border baked in).
    xpad = np.zeros((B, CIN, HP, WP), dtype=np.float32)
    xpad[:, :, 1 : H + 1, 1 : W_ + 1] = x
    # [B, CIN, HP, WP] -> [NT, CIN, B, R+2, WP]
    xt = np.empty((NT, CIN, B, R + 2, WP), dtype=bf16)
    for ht in range(NT):
        xt[ht] = xpad[:, :, ht * R : ht * R + R + 2, :].transpose(1, 0, 2, 3)

    # [cout, cin, kh, kw] -> [cout_chunk, cin, kh*kw, cout_slice], contiguous
    wt = np.ascontiguousarray(
        W.reshape(NCH, 128, CIN, KH * KW).transpose(0, 2, 3, 1), dtype=bf16
    )
    bh = np.ascontiguousarray(b.reshape(NCH, 128).T)

    return [
        {
            "x": np.ascontiguousarray(xt[:, :, core * BPC : (core + 1) * BPC]),
            "wt": wt,
            "bias": bh,
        }
        for core in range(NCORES)
    ]


def kernel(x, W, b):
    from concourse.bass_utils import run_bass_kernel_spmd

    if LDW_SKIP not in _cache:
        _cache[LDW_SKIP] = _build(LDW_SKIP)
    nc = _cache[LDW_SKIP]

    in_maps = _make_in_maps(x, W, b)
    try:
        res = run_bass_kernel_spmd(nc, in_maps, list(range(NCORES))).results
    except Exception:
        # A prior session can leave the accelerator in a transient
        # unrecoverable state; one retry after re-init clears it.
        import time

        time.sleep(15)
        res = run_bass_kernel_spmd(nc, in_maps, list(range(NCORES))).results
    return np.concatenate([res[i]["out"] for i in range(NCORES)], axis=0)


# revision 7
# speedup vs baseline: 1.3555x; 1.1799x over previous
"""Trainium2 Bass kernel for nn_Conv2d: x[32,128,56,56] * W[256,128,3,3] + b -> [32,256,56,56].

Stride 1, padding 1, dilation 1. Data-parallel over batch across 8 NeuronCores
(4 images per core, no collectives).

Per core: 1D Winograd F(2,3) along W. The host transforms the padded input
into 4 components per output-column pair (v0=d0-d2, v1=d1+d2, v2=d2-d1,
v3=d1-d3) and the weights into matching components per vertical tap
(g0=w0, g1=(w0+w1+w2)/2, g2=(w0-w1+w2)/2, g3=w2). On device, each
(row_tile, cout_chunk, image_pair) group runs 12 bf16 matmuls (4 components
x 3 vertical taps, accumulated over taps) into 4 PSUM banks of
[128cout, 2img*8rows*28pairs=448], i.e. 12*448 PE rows per 896 outputs vs
18*448 for direct conv -- a 1.5x tensor-engine reduction. The inverse
transform + bias (y_even = m0+m1+m2+b, y_odd = m1-m2-m3+b) is split across
the scalar (activation with bias/scale), vector, and gpsimd engines and
writes the interleaved output columns, fully hidden under the matmuls.

Self-contained: hardcodes shapes; host does padding/Winograd/bf16 prep so
every device DMA is contiguous.
"""

import numpy as np

B, CIN, H, W_ = 32, 128, 56, 56
COUT, KH, KW = 256, 3, 3
NCORES = 8
BPC = B // NCORES          # images per core
R = 8                      # output rows per tile
NT = H // R                # row tiles per image
HP = H + 2                 # padded rows
J = W_ // 2                # output column pairs
NCOMP = 4                  # Winograd F(2,3) components
NCH = COUT // 128          # cout chunks
NP = BPC // 2              # image pairs per group (matmul free dim 2*R*J=448)

_cache = {}


def _build():
    import concourse.mybir as mybir
    import concourse.tile as tile
    from concourse import bacc

    dt = mybir.dt

    nc = bacc.Bacc("TRN2", target_bir_lowering=False, debug=False)

    # Host-transformed input per row-tile: padded rows ht*R..ht*R+R+1,
    # 4 Winograd components x 28 column pairs.
    v_d = nc.dram_tensor(
        "v", [NT, CIN, BPC, R + 2, NCOMP, J], dt.bfloat16, kind="ExternalInput"
    )
    # Host-transformed weights: [chunk, cin, kh, comp, cout_slice]
    wt_d = nc.dram_tensor(
        "wt", [NCH, CIN, KH, NCOMP, 128], dt.bfloat16, kind="ExternalInput"
    )
    b_d = nc.dram_tensor("bias", [128, NCH], dt.float32, kind="ExternalInput")
    o_d = nc.dram_tensor("out", [BPC, COUT, H, W_], dt.float32, kind="ExternalOutput")

    with tile.TileContext(nc) as tc:
        with (
            tc.tile_pool(name="const", bufs=1) as const_pool,
            tc.tile_pool(name="vin", bufs=1) as vin_pool,
            tc.tile_pool(name="tmp", bufs=8) as tmp_pool,
            tc.tile_pool(name="outp", bufs=4) as out_pool,
            tc.tile_pool(name="psum", bufs=8, space="PSUM") as psum_pool,
        ):
            # All NT row-tiles stay resident (~63KB/partition at bf16).
            vt = {}

            def load_v_tile(ht):
                t = vin_pool.tile(
                    [CIN, BPC, R + 2, NCOMP, J], dt.bfloat16, tag=f"v{ht}"
                )
                vt[ht] = t
                # Split per image: 4 parallel DMA queues shorten the
                # first-group critical path.
                for n in range(BPC):
                    nc.sync.dma_start(t[:, n], v_d[ht, :, n])

            load_v_tile(0)
            w_t = const_pool.tile([CIN, NCH, KH, NCOMP, 128], dt.bfloat16)
            nc.sync.dma_start(w_t[:, 0], wt_d[0])
            nc.sync.dma_start(w_t[:, 1], wt_d[1])
            b_t = const_pool.tile([128, NCH], dt.float32)
            nc.sync.dma_start(b_t[:], b_d[:])
            for ht in range(1, NT):
                load_v_tile(ht)

            for ht in range(NT):
                t = vt[ht]
                for c in range(NCH):
                    for p in range(NP):
                        ps = [
                            psum_pool.tile(
                                [128, 2, R, J], dt.float32, tag="ps", name="ps"
                            )
                            for _ in range(NCOMP)
                        ]
                        for comp in range(NCOMP):
                            for kh in range(KH):
                                nc.tensor.matmul(
                                    ps[comp][:],
                                    w_t[:, c, kh, comp],
                                    t[:, 2 * p : 2 * p + 2, kh : kh + R, comp],
                                    start=(kh == 0),
                                    stop=(kh == KH - 1),
                                )
                        # Inverse transform + bias, split across 3 engines;
                        # every op reads at most one PSUM operand:
                        #   y_even = ((m0 + b) + m1) + m2
                        #   y_odd  = ((b - m3) + m1) - m2
                        s0b = tmp_pool.tile([128, 2, R, J], dt.float32, tag="s0b")
                        nc.scalar.activation(
                            s0b[:],
                            ps[0][:],
                            mybir.ActivationFunctionType.Identity,
                            bias=b_t[:, c : c + 1],
                        )
                        s3n = tmp_pool.tile([128, 2, R, J], dt.float32, tag="s3n")
                        nc.scalar.activation(
                            s3n[:],
                            ps[3][:],
                            mybir.ActivationFunctionType.Identity,
                            bias=b_t[:, c : c + 1],
                            scale=-1.0,
                        )
                        s2 = tmp_pool.tile([128, 2, R, J], dt.float32, tag="s2")
                        nc.scalar.activation(
                            s2[:], ps[2][:], mybir.ActivationFunctionType.Identity
                        )
                        ae = tmp_pool.tile([128, 2, R, J], dt.float32, tag="ae")
                        nc.vector.tensor_add(ae[:], s0b[:], ps[1][:])
                        ao = tmp_pool.tile([128, 2, R, J], dt.float32, tag="ao")
                        nc.vector.tensor_add(ao[:], s3n[:], ps[1][:])
                        # gpsimd cannot access PSUM: it gets the SBUF-only finals
                        ot = out_pool.tile([128, 2, R, W_], dt.float32, tag="ot")
                        nc.gpsimd.tensor_add(ot[:, :, :, 0::2], ae[:], s2[:])
                        nc.gpsimd.tensor_sub(ot[:, :, :, 1::2], ao[:], s2[:])
                        for i in range(2):
                            nc.sync.dma_start(
                                o_d[
                                    2 * p + i,
                                    c * 128 : (c + 1) * 128,
                                    ht * R : ht * R + R,
                                    :,
                                ],
                                ot[:, i],
                            )

    nc.compile()
    return nc


def _make_in_maps(x, W, b):
    import ml_dtypes

    bf16 = ml_dtypes.bfloat16
    x = np.asarray(x, dtype=np.float32)
    W = np.asarray(W, dtype=np.float32)
    b = np.asarray(b, dtype=np.float32)

    # Pad, then 1D Winograd F(2,3) input transform along W (on padded cols):
    # output pair j uses padded cols 2j..2j+3.
    xpad = np.zeros((B, CIN, HP, W_ + 2), dtype=np.float32)
    xpad[:, :, 1 : H + 1, 1 : W_ + 1] = x
    e = xpad[..., 0::2]  # even padded cols 0,2,..,56 (29)
    o = xpad[..., 1::2]  # odd padded cols 1,3,..,57 (29)
    V = np.empty((B, CIN, HP, NCOMP, J), dtype=np.float32)
    V[:, :, :, 0] = e[..., :J] - e[..., 1 : J + 1]   # d0-d2
    V[:, :, :, 1] = o[..., :J] + e[..., 1 : J + 1]   # d1+d2
    V[:, :, :, 2] = e[..., 1 : J + 1] - o[..., :J]   # d2-d1
    V[:, :, :, 3] = o[..., :J] - o[..., 1 : J + 1]   # d1-d3

    # Re-tile: [B, CIN, HP, 4, J] -> [NT, CIN, B, R+2, 4, J]
    vtiles = np.empty((NT, CIN, B, R + 2, NCOMP, J), dtype=bf16)
    for ht in range(NT):
        vtiles[ht] = V[:, :, ht * R : ht * R + R + 2].transpose(1, 0, 2, 3, 4)

    # Weight transform: per kh tap, comps [w0, (w0+w1+w2)/2, (w0-w1+w2)/2, w2]
    w0, w1, w2 = W[..., 0], W[..., 1], W[..., 2]  # each [COUT, CIN, KH]
    g = np.stack(
        [w0, (w0 + w1 + w2) * 0.5, (w0 - w1 + w2) * 0.5, w2], axis=-1
    )  # [COUT, CIN, KH, 4]
    # -> [chunk, cin, kh, comp, cout_slice]
    wt = np.ascontiguousarray(
        g.reshape(NCH, 128, CIN, KH, NCOMP).transpose(0, 2, 3, 4, 1), dtype=bf16
    )
    bh = np.ascontiguousarray(b.reshape(NCH, 128).T)

    return [
        {
            "v": np.ascontiguousarray(vtiles[:, :, core * BPC : (core + 1) * BPC]),
            "wt": wt,
            "bias": bh,
        }
        for core in range(NCORES)
    ]


def kernel(x, W, b):
    from concourse.bass_utils import run_bass_kernel_spmd

    if "nc" not in _cache:
        _cache["nc"] = _build()
    nc = _cache["nc"]

    in_maps = _make_in_maps(x, W, b)
    try:
        res = run_bass_kernel_spmd(nc, in_maps, list(range(NCORES))).results
    except Exception:
        # A prior session can leave the accelerator in a transient
        # unrecoverable state; one retry after re-init clears it.
        import time

        time.sleep(15)
        res = run_bass_kernel_spmd(nc, in_maps, list(range(NCORES))).results
    return np.concatenate([res[i]["out"] for i in range(NCORES)], axis=0)


# revision 11
# speedup vs baseline: 1.5096x; 1.1137x over previous
"""Trainium2 Bass kernel for nn_Conv2d: x[32,128,56,56] * W[256,128,3,3] + b -> [32,256,56,56].

Stride 1, padding 1, dilation 1. Data-parallel over batch across 8 NeuronCores
(4 images per core, no collectives).

Per core: 1D Winograd F(2,3) along W. The host transforms the padded input
into 4 components per output-column pair (v0=d0-d2, v1=d1+d2, v2=d2-d1,
v3=d1-d3) and the weights into matching components per vertical tap
(g0=w0, g1=(w0+w1+w2)/2, g2=(w0-w1+w2)/2, g3=w2). On device, each
(row_tile, cout_chunk, image_pair) group runs 12 bf16 matmuls (4 components
x 3 vertical taps, accumulated over taps) into 4 PSUM banks of
[128cout, 2img*8rows*28pairs=448], i.e. 12*448 PE rows per 896 outputs vs
18*448 for direct conv -- a 1.5x tensor-engine reduction. The inverse
transform + bias (y_even = m0+m1+m2+b, y_odd = m1-m2-m3+b) is split across
the scalar (activation with bias/scale), vector, and gpsimd engines and
writes the interleaved output columns, fully hidden under the matmuls.

Self-contained: hardcodes shapes; host does padding/Winograd/bf16 prep so
every device DMA is contiguous.
"""

import numpy as np

B, CIN, H, W_ = 32, 128, 56, 56
COUT, KH, KW = 256, 3, 3
NCORES = 8
BPC = B // NCORES          # images per core
R = 8                      # output rows per tile
NT = H // R                # row tiles per image
HP = H + 2                 # padded rows
J = W_ // 2                # output column pairs
NCOMP = 4                  # Winograd F(2,3) components
NCH = COUT // 128          # cout chunks
NP = BPC // 2              # image pairs per group (matmul free dim 2*R*J=448)

_cache = {}


def _build():
    import concourse.mybir as mybir
    import concourse.tile as tile
    from concourse import bacc

    dt = mybir.dt

    nc = bacc.Bacc("TRN2", target_bir_lowering=False, debug=False)

    # Host-transformed input per row-tile: padded rows ht*R..ht*R+R+1,
    # 4 Winograd components x 28 column pairs.
    v_d = nc.dram_tensor(
        "v", [NT, CIN, BPC, R + 2, NCOMP, J], dt.bfloat16, kind="ExternalInput"
    )
    # Host-transformed weights: [chunk, cin, kh, comp, cout_slice]
    wt_d = nc.dram_tensor(
        "wt", [NCH, CIN, KH, NCOMP, 128], dt.bfloat16, kind="ExternalInput"
    )
    b_d = nc.dram_tensor("bias", [128, NCH], dt.float32, kind="ExternalInput")
    # fp16 output halves the dominant DMA term (12.8MB -> 6.4MB per core);
    # the host converts back to fp32. Quantization adds ~5e-4 absmax error.
    o_d = nc.dram_tensor("out", [BPC, COUT, H, W_], dt.float16, kind="ExternalOutput")

    with tile.TileContext(nc) as tc:
        with (
            tc.tile_pool(name="const", bufs=1) as const_pool,
            tc.tile_pool(name="vin", bufs=1) as vin_pool,
            tc.tile_pool(name="tmp", bufs=8) as tmp_pool,
            tc.tile_pool(name="outp", bufs=4) as out_pool,
            tc.tile_pool(name="psum", bufs=8, space="PSUM") as psum_pool,
        ):
            # All NT row-tiles stay resident (~63KB/partition at bf16).
            vt = {}

            def load_v_tile(ht):
                t = vin_pool.tile(
                    [CIN, BPC, R + 2, NCOMP, J], dt.bfloat16, tag=f"v{ht}"
                )
                vt[ht] = t
                # Split per image: 4 parallel DMA queues shorten the
                # first-group critical path.
                for n in range(BPC):
                    nc.sync.dma_start(t[:, n], v_d[ht, :, n])

            load_v_tile(0)
            w_t = const_pool.tile([CIN, NCH, KH, NCOMP, 128], dt.bfloat16)
            nc.sync.dma_start(w_t[:, 0], wt_d[0])
            nc.sync.dma_start(w_t[:, 1], wt_d[1])
            b_t = const_pool.tile([128, NCH], dt.float32)
            nc.sync.dma_start(b_t[:], b_d[:])
            for ht in range(1, NT):
                load_v_tile(ht)

            # Output staging: per (c, image-pair) buffer holding TWO row-tiles
            # (16 rows) in fp16, flushed as one DMA per image with 1792B
            # per-partition descriptors (vs 896B if flushed per row-tile).
            ot_buf = {}

            for ht in range(NT):
                t = vt[ht]
                for c in range(NCH):
                    for p in range(NP):
                        ps = [
                            psum_pool.tile(
                                [128, 2, R, J], dt.float32, tag="ps", name="ps"
                            )
                            for _ in range(NCOMP)
                        ]
                        for comp in range(NCOMP):
                            for kh in range(KH):
                                nc.tensor.matmul(
                                    ps[comp][:],
                                    w_t[:, c, kh, comp],
                                    t[:, 2 * p : 2 * p + 2, kh : kh + R, comp],
                                    start=(kh == 0),
                                    stop=(kh == KH - 1),
                                )
                        # Inverse transform + bias, split across 3 engines;
                        # every op reads at most one PSUM operand:
                        #   y_even = ((m0 + b) + m1) + m2
                        #   y_odd  = ((b - m3) + m1) - m2
                        s0b = tmp_pool.tile([128, 2, R, J], dt.float32, tag="s0b")
                        nc.scalar.activation(
                            s0b[:],
                            ps[0][:],
                            mybir.ActivationFunctionType.Identity,
                            bias=b_t[:, c : c + 1],
                        )
                        s3n = tmp_pool.tile([128, 2, R, J], dt.float32, tag="s3n")
                        nc.scalar.activation(
                            s3n[:],
                            ps[3][:],
                            mybir.ActivationFunctionType.Identity,
                            bias=b_t[:, c : c + 1],
                            scale=-1.0,
                        )
                        s2 = tmp_pool.tile([128, 2, R, J], dt.float32, tag="s2")
                        nc.scalar.activation(
                            s2[:], ps[2][:], mybir.ActivationFunctionType.Identity
                        )
                        ae = tmp_pool.tile([128, 2, R, J], dt.float32, tag="ae")
                        nc.vector.tensor_add(ae[:], s0b[:], ps[1][:])
                        ao = tmp_pool.tile([128, 2, R, J], dt.float32, tag="ao")
                        nc.vector.tensor_add(ao[:], s3n[:], ps[1][:])
                        # gpsimd cannot access PSUM: it gets the SBUF-only finals
                        if ht % 2 == 0:
                            ot_buf[(c, p)] = out_pool.tile(
                                [128, 2, 2, R, W_], dt.float16, tag="ot", name="ot"
                            )
                        ot = ot_buf[(c, p)]
                        par = ht % 2
                        nc.gpsimd.tensor_add(ot[:, :, par, :, 0::2], ae[:], s2[:])
                        nc.gpsimd.tensor_sub(ot[:, :, par, :, 1::2], ao[:], s2[:])
                        if par == 1 or ht == NT - 1:
                            h0 = (ht - par) * R
                            nrows = (par + 1) * R
                            for i in range(2):
                                nc.sync.dma_start(
                                    o_d[
                                        2 * p + i,
                                        c * 128 : (c + 1) * 128,
                                        h0 : h0 + nrows,
                                        :,
                                    ],
                                    ot[:, i, : par + 1],
                                )

    nc.compile()
    return nc


def _make_in_maps(x, W, b):
    import ml_dtypes

    bf16 = ml_dtypes.bfloat16
    x = np.asarray(x, dtype=np.float32)
    W = np.asarray(W, dtype=np.float32)
    b = np.asarray(b, dtype=np.float32)

    # Pad, then 1D Winograd F(2,3) input transform along W (on padded cols):
    # output pair j uses padded cols 2j..2j+3.
    xpad = np.zeros((B, CIN, HP, W_ + 2), dtype=np.float32)
    xpad[:, :, 1 : H + 1, 1 : W_ + 1] = x
    e = xpad[..., 0::2]  # even padded cols 0,2,..,56 (29)
    o = xpad[..., 1::2]  # odd padded cols 1,3,..,57 (29)
    V = np.empty((B, CIN, HP, NCOMP, J), dtype=np.float32)
    V[:, :, :, 0] = e[..., :J] - e[..., 1 : J + 1]   # d0-d2
    V[:, :, :, 1] = o[..., :J] + e[..., 1 : J + 1]   # d1+d2
    V[:, :, :, 2] = e[..., 1 : J + 1] - o[..., :J]   # d2-d1
    V[:, :, :, 3] = o[..., :J] - o[..., 1 : J + 1]   # d1-d3

    # Re-tile: [B, CIN, HP, 4, J] -> [NT, CIN, B, R+2, 4, J]
    vtiles = np.empty((NT, CIN, B, R + 2, NCOMP, J), dtype=bf16)
    for ht in range(NT):
        vtiles[ht] = V[:, :, ht * R : ht * R + R + 2].transpose(1, 0, 2, 3, 4)

    # Weight transform: per kh tap, comps [w0, (w0+w1+w2)/2, (w0-w1+w2)/2, w2]
    w0, w1, w2 = W[..., 0], W[..., 1], W[..., 2]  # each [COUT, CIN, KH]
    g = np.stack(
        [w0, (w0 + w1 + w2) * 0.5, (w0 - w1 + w2) * 0.5, w2], axis=-1
    )  # [COUT, CIN, KH, 4]
    # -> [chunk, cin, kh, comp, cout_slice]
    wt = np.ascontiguousarray(
        g.reshape(NCH, 128, CIN, KH, NCOMP).transpose(0, 2, 3, 4, 1), dtype=bf16
    )
    bh = np.ascontiguousarray(b.reshape(NCH, 128).T)

    return [
        {
            "v": np.ascontiguousarray(vtiles[:, :, core * BPC : (core + 1) * BPC]),
            "wt": wt,
            "bias": bh,
        }
        for core in range(NCORES)
    ]


def kernel(x, W, b):
    from concourse.bass_utils import run_bass_kernel_spmd

    if "nc" not in _cache:
        _cache["nc"] = _build()
    nc = _cache["nc"]

    in_maps = _make_in_maps(x, W, b)
    try:
        res = run_bass_kernel_spmd(nc, in_maps, list(range(NCORES))).results
    except Exception:
        # A prior session can leave the accelerator in a transient
        # unrecoverable state; one retry after re-init clears it.
        import time

        time.sleep(15)
        res = run_bass_kernel_spmd(nc, in_maps, list(range(NCORES))).results
    return np.concatenate(
        [res[i]["out"].astype(np.float32) for i in range(NCORES)], axis=0
    )
